# revision 6
# baseline (speedup 1.0000x reference)
"""Gaussian upsampling (https://arxiv.org/abs/2010.04301) on 8 trn2 NeuronCores.

out[b, t, :] = softmax_j(-DELTA * (t - c_j)^2) @ hs[b, :, :],
c = cumsum(ds) - ds/2.

Key structure: with DELTA = 0.1 the Gaussian weight exp(-0.1 d^2)
underflows to exactly 0 in fp32 for |d| > ~33 frames, so softmax rows are
banded: each 128-frame block of output only sees a narrow window of
tokens.  We gather per-block token windows on the host (index prep only),
and on-device compute energies, exp, softmax normalization and the
weighted sum with hs.

Like the reference's softmax we subtract the per-frame max energy
(ds.sum() < T_FEATS here, so most frames lie beyond the last token
center and their whole energy row is hugely negative — without the
shift every weight underflows and softmax is 0/0).  The shift
(t - c_nearest)^2 is pure index math, precomputed on host, and makes
the nearest token's weight exactly 1, so sums stay in [1, ~2.6].

Sharding: core = b * 4 + q handles batch b, frames [4096 q, 4096 (q+1)).
Within a core, frames are tiled in 128-frame blocks; 4 blocks (one
"superblock") share a [128, 128] SBUF tile of exp-energies laid out as
4 groups x W=32 window tokens on partitions, 128 relative frames on the
free axis.  Per superblock:
  - ScalarE: y = Square(t_rel + (t0 - c_j))   (bias per partition)
  - DVE:     y -= shift                        (per-frame max energy)
  - ScalarE: u = Exp(-DELTA * y)              (unnormalized weights)
  - split u = uh + ul (two bf16 halves) so the PE runs bf16 at full rate
    while keeping ~fp32 accuracy: u @ w ~= uh@wh + uh@wl + ul@wh
    (the dropped ul@wl term is ~4e-6 relative)
  - PE:      sums[f, g] = (uh+ul).T @ G       (G = group indicator)
  - DVE:     r = 1 / sums
  - PE:      per block g: 3 accumulated row-tiled K=32 bf16 matmuls
  - DVE/ACT: out_g = psum_g * r[:, g]  (alternating engines)
  - one DMA per superblock writes 4 normalized blocks (1 MiB).
"""

import os

import ml_dtypes
import numpy as np

import concourse.bacc as bacc
import concourse.mybir as mybir
import concourse.tile as tile
from concourse.bass_utils import run_bass_kernel_spmd

DELTA = 0.1
B = 2
T_TEXT = 1024
ADIM = 512
T_FEATS = 16384
N_CORES = 8
Q_PER_B = N_CORES // B           # frame-quarters per batch
F_CORE = T_FEATS // Q_PER_B      # frames per core (4096)
FB = 128                         # frames per block
NBLK = F_CORE // FB              # blocks per core (32)
W = 32                           # token window per block
GRP = 128 // W                   # blocks per superblock (4)
NSUP = NBLK // GRP               # superblocks per core (8)
# tokens farther than this from every frame of a block contribute exactly
# 0 in fp32 (exp underflow at |d| ~ 33); 45 leaves margin.
REACH = 45.0

BF16 = ml_dtypes.bfloat16

_LAST_EXEC_NS = None


def _build_program():
    nc = bacc.Bacc(
        "TRN2", target_bir_lowering=False, debug=False, num_devices=N_CORES
    )
    f32 = mybir.dt.float32
    bf16 = mybir.dt.bfloat16

    wh_d = nc.dram_tensor("wh", [NSUP, 128, ADIM], bf16, kind="ExternalInput").ap()
    wl_d = nc.dram_tensor("wl", [NSUP, 128, ADIM], bf16, kind="ExternalInput").ap()
    negc = nc.dram_tensor("negc", [128, NSUP], f32, kind="ExternalInput").ap()
    shift = nc.dram_tensor("shift", [NSUP, 128, FB], f32, kind="ExternalInput").ap()
    tt = nc.dram_tensor("tt", [128, FB], f32, kind="ExternalInput").ap()
    gmat = nc.dram_tensor("gmat", [128, GRP], bf16, kind="ExternalInput").ap()
    out = nc.dram_tensor("out", [F_CORE, ADIM], f32, kind="ExternalOutput").ap()
    # DRAM view of out grouped per superblock: (p, g, d)
    out_sb = out.rearrange("(s g p) d -> s p g d", g=GRP, p=FB)

    Act = mybir.ActivationFunctionType

    with tile.TileContext(nc) as tc:
        with (
            tc.tile_pool(name="const", bufs=1) as const_pool,
            tc.tile_pool(name="win", bufs=3) as win_pool,
            tc.tile_pool(name="u", bufs=3) as u_pool,
            tc.tile_pool(name="ob", bufs=2) as out_pool,
            tc.tile_pool(name="rc", bufs=2) as rc_pool,
            tc.tile_pool(name="ps_m", bufs=4, space="PSUM") as psm_pool,
            tc.tile_pool(name="ps_s", bufs=2, space="PSUM") as pss_pool,
        ):
            tt_t = const_pool.tile([128, FB], f32)
            nc.sync.dma_start(out=tt_t, in_=tt)
            g_t = const_pool.tile([128, GRP], bf16)
            nc.sync.dma_start(out=g_t, in_=gmat)
            negc_t = const_pool.tile([128, NSUP], f32)
            nc.sync.dma_start(out=negc_t, in_=negc)

            for s in range(NSUP):
                wh_t = win_pool.tile([128, ADIM], bf16, tag="wh")
                nc.sync.dma_start(out=wh_t, in_=wh_d[s])
                wl_t = win_pool.tile([128, ADIM], bf16, tag="wl")
                nc.sync.dma_start(out=wl_t, in_=wl_d[s])
                sh_t = win_pool.tile([128, FB], f32, tag="sh")
                nc.sync.dma_start(out=sh_t, in_=shift[s])

                u_t = u_pool.tile([128, FB], f32, tag="uf")
                # y = (t_rel + (t0 - c_j))^2
                nc.scalar.activation(
                    u_t, tt_t, Act.Square, bias=negc_t[:, s : s + 1], scale=1.0
                )
                # y -= (t - c_nearest)^2  (per-frame max-energy shift)
                nc.vector.tensor_sub(u_t, u_t, sh_t)
                # u = exp(-DELTA * y)
                nc.scalar.activation(u_t, u_t, Act.Exp, scale=-DELTA)
                uh_t = u_pool.tile([128, FB], bf16, tag="uh")
                nc.vector.tensor_copy(uh_t, u_t)
                ul_t = u_pool.tile([128, FB], bf16, tag="ul")
                nc.vector.tensor_sub(ul_t, u_t, uh_t)

                ps = pss_pool.tile([128, GRP], f32)
                nc.tensor.matmul(ps, lhsT=uh_t, rhs=g_t, start=True, stop=False)
                nc.tensor.matmul(ps, lhsT=ul_t, rhs=g_t, start=False, stop=True)
                r_t = rc_pool.tile([128, GRP], f32)
                nc.vector.reciprocal(r_t, ps)

                ob = out_pool.tile([128, GRP, ADIM], f32)
                for g in range(GRP):
                    blk = s * GRP + g
                    sl = slice(g * W, (g + 1) * W)
                    tp = (g * W, 0)
                    pm = psm_pool.tile([128, ADIM], f32)
                    nc.tensor.matmul(
                        pm, lhsT=uh_t[sl, :], rhs=wh_t[sl, :],
                        start=True, stop=False, tile_position=tp,
                    )
                    nc.tensor.matmul(
                        pm, lhsT=uh_t[sl, :], rhs=wl_t[sl, :],
                        start=False, stop=False, tile_position=tp,
                    )
                    nc.tensor.matmul(
                        pm, lhsT=ul_t[sl, :], rhs=wh_t[sl, :],
                        start=False, stop=True, tile_position=tp,
                    )
                    if blk % 2 == 0:
                        nc.scalar.activation(
                            ob[:, g, :], pm, Act.Copy, scale=r_t[:, g : g + 1]
                        )
                    else:
                        nc.vector.tensor_scalar_mul(
                            ob[:, g, :], pm, r_t[:, g : g + 1]
                        )
                nc.sync.dma_start(out=out_sb[s], in_=ob)

    nc.compile()
    return nc


def _host_prep(hs, ds):
    """Per-core input maps: gathered hs windows + energy-bias tables."""
    hs = np.asarray(hs, dtype=np.float32)
    ds = np.asarray(ds)
    in_maps = []
    ttab = np.tile(np.arange(FB, dtype=np.float32), (128, 1))
    gm = np.zeros((128, GRP), dtype=BF16)
    for g in range(GRP):
        gm[g * W : (g + 1) * W, g] = 1.0
    for b in range(B):
        ds_f = ds[b].astype(np.float32)
        c = np.cumsum(ds_f) - ds_f / 2.0  # token centers, fp32 as in reference
        # nearest center per output frame (for the max-energy shift)
        t_all = np.arange(T_FEATS, dtype=np.float32)
        ins_pt = np.searchsorted(c, t_all)
        cand_lo = np.clip(ins_pt - 1, 0, T_TEXT - 1)
        cand_hi = np.clip(ins_pt, 0, T_TEXT - 1)
        pick_hi = np.abs(c[cand_hi] - t_all) < np.abs(c[cand_lo] - t_all)
        near = np.where(pick_hi, cand_hi, cand_lo)
        d2 = (t_all - c[near]) ** 2  # fp32
        for q in range(Q_PER_B):
            hs_win = np.zeros((NSUP, 128, ADIM), dtype=np.float32)
            negc = np.zeros((128, NSUP), dtype=np.float32)
            shift = np.zeros((NSUP, 128, FB), dtype=np.float32)
            for s in range(NSUP):
                for g in range(GRP):
                    gi = q * NBLK + s * GRP + g  # global block in this batch
                    t0 = gi * FB
                    lo = int(np.searchsorted(c, t0 - REACH, side="left"))
                    hi = int(np.searchsorted(c, t0 + (FB - 1) + REACH, side="right"))
                    n_lo = int(near[t0 : t0 + FB].min())
                    n_hi = int(near[t0 : t0 + FB].max())
                    j0 = max(0, min(lo, n_lo, T_TEXT - W))
                    assert max(hi, n_hi + 1) - j0 <= W, (
                        f"token window {max(hi, n_hi + 1) - j0} exceeds {W}; "
                        "durations too small for this kernel's banding"
                    )
                    hs_win[s, g * W : (g + 1) * W, :] = hs[b, j0 : j0 + W, :]
                    negc[g * W : (g + 1) * W, s] = t0 - c[j0 : j0 + W]
                    shift[s, g * W : (g + 1) * W, :] = d2[t0 : t0 + FB]
            wh = hs_win.astype(BF16)
            wl = (hs_win - wh.astype(np.float32)).astype(BF16)
            in_maps.append(
                {
                    "wh": wh, "wl": wl, "negc": negc, "shift": shift,
                    "tt": ttab, "gmat": gm,
                }
            )
    return in_maps


def kernel(hs, ds):
    global _LAST_EXEC_NS
    in_maps = _host_prep(hs, ds)
    nc = _build_program()

    kwargs = {}
    if os.environ.get("GU_TRACE") == "1":
        import concourse.bass_utils as bu

        bu.upload_artifacts = lambda tmpdir: "local://" + tmpdir
        kwargs = {"trace": True}
    res = run_bass_kernel_spmd(nc, in_maps, list(range(N_CORES)), **kwargs)
    _LAST_EXEC_NS = res.exec_time_ns

    full = np.empty((B, T_FEATS, ADIM), dtype=np.float32)
    for b in range(B):
        for q in range(Q_PER_B):
            core = b * Q_PER_B + q
            full[b, q * F_CORE : (q + 1) * F_CORE, :] = res.results[core]["out"]
    return full


# revision 7
# speedup vs baseline: 1.0040x; 1.0040x over previous
"""Gaussian upsampling (https://arxiv.org/abs/2010.04301) on 8 trn2 NeuronCores.

out[b, t, :] = softmax_j(-DELTA * (t - c_j)^2) @ hs[b, :, :],
c = cumsum(ds) - ds/2.

Key structure: with DELTA = 0.1 the Gaussian weight exp(-0.1 d^2)
underflows to exactly 0 in fp32 for |d| > ~33 frames, so softmax rows are
banded: each 128-frame block of output only sees a narrow window of
tokens.  We gather per-block token windows on the host (index prep only),
and on-device compute energies, exp, softmax normalization and the
weighted sum with hs.

Like the reference's softmax we subtract the per-frame max energy
(ds.sum() < T_FEATS here, so most frames lie beyond the last token
center and their whole energy row is hugely negative — without the
shift every weight underflows and softmax is 0/0).  The shift
(t - c_nearest)^2 is pure index math, precomputed on host, and makes
the nearest token's weight exactly 1, so sums stay in [1, ~2.6].

Precision: the PE runs f16 (full rate).  The attention weights u lie in
[0, 1] where f16's 11-bit mantissa gives ~5e-4 relative error that the
softmax normalization mostly cancels; hs is split into f16 high + low
halves (u @ w = u16 @ wh + u16 @ wl) which removes the hs rounding
entirely.  Measured ~1e-5 scale-relative output error vs the fp32
reference.

Sharding: core = b * 4 + q handles batch b, frames [4096 q, 4096 (q+1)).
Within a core, frames are tiled in 128-frame blocks; 4 blocks (one
"superblock") share a [128, 128] SBUF tile of exp-energies laid out as
4 groups x W=32 window tokens on partitions, 128 relative frames on the
free axis.  Per superblock:
  - one packed DMA brings wh | wl | shift (contiguous u8, bitcast views)
  - ScalarE: y = Square(t_rel + (t0 - c_j))   (bias per partition)
  - DVE:     y -= shift                        (per-frame max energy)
  - ScalarE: u = Exp(-DELTA * y);  DVE cast to f16
  - PE:      sums[f, g] = u16.T @ G            (G = group indicator)
  - DVE:     r = 1 / sums
  - PE:      per block g: 2 accumulated row-tiled K=32 f16 matmuls
  - DVE/ACT: out_g = psum_g * r[:, g]  (alternating engines)
Output is written in blocked [superblock, frame_rel, block, adim] layout
with one contiguous 2 MiB DMA per superblock pair; host un-permutes.
"""

import os

import ml_dtypes
import numpy as np

import concourse.bacc as bacc
import concourse.mybir as mybir
import concourse.tile as tile
from concourse.bass_utils import run_bass_kernel_spmd

DELTA = 0.1
B = 2
T_TEXT = 1024
ADIM = 512
T_FEATS = 16384
N_CORES = 8
Q_PER_B = N_CORES // B           # frame-quarters per batch
F_CORE = T_FEATS // Q_PER_B      # frames per core (4096)
FB = 128                         # frames per block
NBLK = F_CORE // FB              # blocks per core (32)
W = 32                           # token window per block
GRP = 128 // W                   # blocks per superblock (4)
NSUP = NBLK // GRP               # superblocks per core (8)
# tokens farther than this from every frame of a block contribute exactly
# 0 in fp32 (exp underflow at |d| ~ 33); 45 leaves margin.
REACH = 45.0

# packed per-superblock input bytes per partition:
#   [0:1024)    wh  f16[512]
#   [1024:2048) wl  f16[512]
#   [2048:2560) sh  f32[128]
WIN_BYTES = 2560
# packed consts per partition: tt f32[128] @ 0, negc f32[NSUP] @ 512,
# gmat f16[GRP] @ 512 + 4*NSUP
CON_TT, CON_NEGC, CON_G = 0, 512, 512 + 4 * NSUP
CON_BYTES = CON_G + 2 * GRP

_LAST_EXEC_NS = None


def _build_program():
    nc = bacc.Bacc(
        "TRN2", target_bir_lowering=False, debug=False, num_devices=N_CORES
    )
    f32 = mybir.dt.float32
    f16 = mybir.dt.float16
    u8 = mybir.dt.uint8

    win_d = nc.dram_tensor("win", [NSUP, 128, WIN_BYTES], u8, kind="ExternalInput").ap()
    con_d = nc.dram_tensor("con", [128, CON_BYTES], u8, kind="ExternalInput").ap()
    out = nc.dram_tensor(
        "out", [NSUP // 2, 128, 2 * GRP * ADIM], f32, kind="ExternalOutput"
    ).ap()

    Act = mybir.ActivationFunctionType

    with tile.TileContext(nc) as tc:
        with (
            tc.tile_pool(name="const", bufs=1) as const_pool,
            tc.tile_pool(name="win", bufs=3) as win_pool,
            tc.tile_pool(name="u", bufs=2) as u_pool,
            tc.tile_pool(name="ob", bufs=2) as out_pool,
            tc.tile_pool(name="rc", bufs=2) as rc_pool,
            tc.tile_pool(name="ps_m", bufs=6, space="PSUM") as psm_pool,
            tc.tile_pool(name="ps_s", bufs=2, space="PSUM") as pss_pool,
        ):
            con_t = const_pool.tile([128, CON_BYTES], u8)
            nc.sync.dma_start(out=con_t, in_=con_d)
            tt_v = con_t[:, CON_TT : CON_TT + 512].bitcast(f32)      # [128, 128]
            negc_v = con_t[:, CON_NEGC : CON_NEGC + 4 * NSUP].bitcast(f32)
            g_v = con_t[:, CON_G : CON_G + 2 * GRP].bitcast(f16)     # [128, GRP]

            for s in range(NSUP):
                wt = win_pool.tile([128, WIN_BYTES], u8)
                nc.sync.dma_start(out=wt, in_=win_d[s])
                wh_v = wt[:, 0:1024].bitcast(f16)          # [128, 512]
                wl_v = wt[:, 1024:2048].bitcast(f16)       # [128, 512]
                sh_v = wt[:, 2048:2560].bitcast(f32)       # [128, 128]

                u_t = u_pool.tile([128, FB], f32, tag="uf")
                # y = (t_rel + (t0 - c_j))^2
                nc.scalar.activation(
                    u_t, tt_v, Act.Square, bias=negc_v[:, s : s + 1], scale=1.0
                )
                # y -= (t - c_nearest)^2  (per-frame max-energy shift)
                nc.vector.tensor_sub(u_t, u_t, sh_v)
                # u = exp(-DELTA * y)
                nc.scalar.activation(u_t, u_t, Act.Exp, scale=-DELTA)
                u16_t = u_pool.tile([128, FB], f16, tag="u16")
                nc.vector.tensor_copy(u16_t, u_t)

                ps = pss_pool.tile([128, GRP], f32)
                nc.tensor.matmul(ps, lhsT=u16_t, rhs=g_v, start=True, stop=True)
                r_t = rc_pool.tile([128, GRP], f32)
                nc.vector.reciprocal(r_t, ps)

                if s % 2 == 0:
                    ob = out_pool.tile([128, 2 * GRP * ADIM], f32)
                for g in range(GRP):
                    blk = s * GRP + g
                    sl = slice(g * W, (g + 1) * W)
                    tp = (g * W, 0)
                    pm = psm_pool.tile([128, ADIM], f32)
                    nc.tensor.matmul(
                        pm, lhsT=u16_t[sl, :], rhs=wh_v[sl, :],
                        start=True, stop=False, tile_position=tp,
                    )
                    nc.tensor.matmul(
                        pm, lhsT=u16_t[sl, :], rhs=wl_v[sl, :],
                        start=False, stop=True, tile_position=tp,
                    )
                    col = ((s % 2) * GRP + g) * ADIM
                    dst = ob[:, col : col + ADIM]
                    if blk % 2 == 0:
                        nc.scalar.activation(
                            dst, pm, Act.Copy, scale=r_t[:, g : g + 1]
                        )
                    else:
                        nc.vector.tensor_scalar_mul(dst, pm, r_t[:, g : g + 1])
                if s % 2 == 1:
                    nc.sync.dma_start(out=out[s // 2], in_=ob)

    nc.compile()
    return nc


def _host_prep(hs, ds):
    """Per-core input maps: packed gathered hs windows + bias tables."""
    hs = np.asarray(hs, dtype=np.float32)
    ds = np.asarray(ds)
    in_maps = []
    ttab = np.tile(np.arange(FB, dtype=np.float32), (128, 1))
    gm = np.zeros((128, GRP), dtype=np.float16)
    for g in range(GRP):
        gm[g * W : (g + 1) * W, g] = 1.0
    for b in range(B):
        ds_f = ds[b].astype(np.float32)
        c = np.cumsum(ds_f) - ds_f / 2.0  # token centers, fp32 as in reference
        # nearest center per output frame (for the max-energy shift)
        t_all = np.arange(T_FEATS, dtype=np.float32)
        ins_pt = np.searchsorted(c, t_all)
        cand_lo = np.clip(ins_pt - 1, 0, T_TEXT - 1)
        cand_hi = np.clip(ins_pt, 0, T_TEXT - 1)
        pick_hi = np.abs(c[cand_hi] - t_all) < np.abs(c[cand_lo] - t_all)
        near = np.where(pick_hi, cand_hi, cand_lo)
        d2 = (t_all - c[near]) ** 2  # fp32
        for q in range(Q_PER_B):
            hs_win = np.zeros((NSUP, 128, ADIM), dtype=np.float32)
            negc = np.zeros((128, NSUP), dtype=np.float32)
            shift = np.zeros((NSUP, 128, FB), dtype=np.float32)
            for s in range(NSUP):
                for g in range(GRP):
                    gi = q * NBLK + s * GRP + g  # global block in this batch
                    t0 = gi * FB
                    lo = int(np.searchsorted(c, t0 - REACH, side="left"))
                    hi = int(np.searchsorted(c, t0 + (FB - 1) + REACH, side="right"))
                    n_lo = int(near[t0 : t0 + FB].min())
                    n_hi = int(near[t0 : t0 + FB].max())
                    j0 = max(0, min(lo, n_lo, T_TEXT - W))
                    assert max(hi, n_hi + 1) - j0 <= W, (
                        f"token window {max(hi, n_hi + 1) - j0} exceeds {W}; "
                        "durations too small for this kernel's banding"
                    )
                    hs_win[s, g * W : (g + 1) * W, :] = hs[b, j0 : j0 + W, :]
                    negc[g * W : (g + 1) * W, s] = t0 - c[j0 : j0 + W]
                    shift[s, g * W : (g + 1) * W, :] = d2[t0 : t0 + FB]
            wh = hs_win.astype(np.float16)
            wl = (hs_win - wh.astype(np.float32)).astype(np.float16)
            win = np.empty((NSUP, 128, WIN_BYTES), dtype=np.uint8)
            win[:, :, 0:1024] = wh.view(np.uint8)
            win[:, :, 1024:2048] = wl.view(np.uint8)
            win[:, :, 2048:2560] = shift.view(np.uint8)
            con = np.empty((128, CON_BYTES), dtype=np.uint8)
            con[:, CON_TT : CON_TT + 512] = ttab.view(np.uint8)
            con[:, CON_NEGC : CON_NEGC + 4 * NSUP] = negc.view(np.uint8)
            con[:, CON_G : CON_G + 2 * GRP] = gm.view(np.uint8)
            in_maps.append({"win": win, "con": con})
    return in_maps


def kernel(hs, ds):
    global _LAST_EXEC_NS
    in_maps = _host_prep(hs, ds)
    nc = _build_program()

    kwargs = {}
    if os.environ.get("GU_TRACE") == "1":
        import concourse.bass_utils as bu

        bu.upload_artifacts = lambda tmpdir: "local://" + tmpdir
        kwargs = {"trace": True}
    res = run_bass_kernel_spmd(nc, in_maps, list(range(N_CORES)), **kwargs)
    _LAST_EXEC_NS = res.exec_time_ns

    full = np.empty((B, T_FEATS, ADIM), dtype=np.float32)
    for b in range(B):
        for q in range(Q_PER_B):
            core = b * Q_PER_B + q
            blocked = res.results[core]["out"]  # [NSUP//2, 128, 2*GRP*ADIM]
            o = blocked.reshape(NSUP // 2, 128, 2 * GRP, ADIM)
            o = o.transpose(0, 2, 1, 3).reshape(F_CORE, ADIM)
            full[b, q * F_CORE : (q + 1) * F_CORE, :] = o
    return full


# revision 11
# speedup vs baseline: 1.1965x; 1.1917x over previous
"""Gaussian upsampling (https://arxiv.org/abs/2010.04301) on 8 trn2 NeuronCores.

out[b, t, :] = softmax_j(-DELTA * (t - c_j)^2) @ hs[b, :, :],
c = cumsum(ds) - ds/2.

Key structure: with DELTA = 0.1 the Gaussian weight exp(-0.1 d^2)
underflows to exactly 0 in fp32 for |d| > ~33 frames, so softmax rows are
banded: each 128-frame block of output only sees a narrow window of
tokens.  We gather per-block token windows on the host (index prep only),
and on-device compute energies, exp, softmax normalization and the
weighted sum with hs.

Like the reference's softmax we subtract the per-frame max energy
(ds.sum() < T_FEATS here, so most frames lie beyond the last token
center and their whole energy row is hugely negative — without the
shift every weight underflows and softmax is 0/0).  The shift
(t - c_nearest)^2 is pure index math, precomputed on host, and makes
the nearest token's weight exactly 1, so sums stay in [1, ~2.6].

Precision: the PE runs f16 (full rate).  The attention weights u lie in
[0, 1] where f16's 11-bit mantissa gives ~5e-4 relative error that the
softmax normalization mostly cancels; hs is split into f16 high + low
halves (u @ w = u16 @ wh + u16 @ wl) which removes the hs rounding
entirely.  Measured ~1e-5 scale-relative output error vs the fp32
reference.

Sharding: core = b * 4 + q handles batch b, frames [4096 q, 4096 (q+1)).
Within a core, frames are tiled in 128-frame blocks; 4 blocks (one
"superblock") share a [128, 128] SBUF tile of exp-energies laid out as
4 groups x W=32 window tokens on partitions, 128 relative frames on the
free axis.  Per superblock:
  - one packed DMA brings wh | wl | shift (contiguous u8, bitcast views)
  - ScalarE: y = Square(t_rel + (t0 - c_j))   (bias per partition)
  - DVE:     y -= shift                        (per-frame max energy)
  - ScalarE: u = Exp(-DELTA * y);  DVE cast to f16
  - PE:      sums[f, g] = u16.T @ G            (G = group indicator)
  - DVE:     r = 1 / sums
  - PE:      per block g: 2 accumulated row-tiled K=32 f16 matmuls
  - DVE/ACT: out_g = psum_g * r[:, g]  (alternating engines)
Output is written in blocked [superblock, frame_rel, block, adim] layout
with one contiguous 2 MiB DMA per superblock pair; host un-permutes.
"""

import os

import ml_dtypes
import numpy as np

import concourse.bacc as bacc
import concourse.mybir as mybir
import concourse.tile as tile
from concourse.bass_utils import run_bass_kernel_spmd

DELTA = 0.1
B = 2
T_TEXT = 1024
ADIM = 512
T_FEATS = 16384
N_CORES = 8
Q_PER_B = N_CORES // B           # frame-quarters per batch
F_CORE = T_FEATS // Q_PER_B      # frames per core (4096)
FB = 128                         # frames per block
NBLK = F_CORE // FB              # blocks per core (32)
W = 32                           # token window per block
GRP = 128 // W                   # blocks per superblock (4)
NSUP = NBLK // GRP               # superblocks per core (8)
# tokens farther than this from every frame of a block contribute exactly
# 0 in fp32 (exp underflow at |d| ~ 33); 45 leaves margin.
REACH = 45.0

# packed per-superblock input bytes per partition:
#   [0:1024)    wh  f16[512]
#   [1024:2048) wl  f16[512]
#   [2048:2560) sh  f32[128]
WIN_BYTES = 2560
# packed consts per partition: tt f32[128] @ 0, negc f32[NSUP] @ 512,
# gmat f16[GRP] @ 512 + 4*NSUP
CON_TT, CON_NEGC, CON_G = 0, 512, 512 + 4 * NSUP
CON_BYTES = CON_G + 2 * GRP

_LAST_EXEC_NS = None


def _build_program():
    nc = bacc.Bacc(
        "TRN2", target_bir_lowering=False, debug=False, num_devices=N_CORES
    )
    f32 = mybir.dt.float32
    f16 = mybir.dt.float16
    u8 = mybir.dt.uint8

    win_d = nc.dram_tensor("win", [NSUP, 128, WIN_BYTES], u8, kind="ExternalInput").ap()
    con_d = nc.dram_tensor("con", [128, CON_BYTES], u8, kind="ExternalInput").ap()
    out = nc.dram_tensor(
        "out", [NSUP, 128, GRP * ADIM], f32, kind="ExternalOutput"
    ).ap()

    Act = mybir.ActivationFunctionType

    with tile.TileContext(nc) as tc:
        with (
            tc.tile_pool(name="const", bufs=1) as const_pool,
            tc.tile_pool(name="win", bufs=NSUP) as win_pool,
            tc.tile_pool(name="u", bufs=2) as u_pool,
            tc.tile_pool(name="ob", bufs=2) as out_pool,
            tc.tile_pool(name="rc", bufs=2) as rc_pool,
            tc.tile_pool(name="ps_m", bufs=6, space="PSUM") as psm_pool,
            tc.tile_pool(name="ps_s", bufs=2, space="PSUM") as pss_pool,
        ):
            con_t = const_pool.tile([128, CON_BYTES], u8)
            nc.sync.dma_start(out=con_t, in_=con_d)
            tt_v = con_t[:, CON_TT : CON_TT + 512].bitcast(f32)      # [128, 128]
            negc_v = con_t[:, CON_NEGC : CON_NEGC + 4 * NSUP].bitcast(f32)
            g_v = con_t[:, CON_G : CON_G + 2 * GRP].bitcast(f16)     # [128, GRP]

            # prefetch every superblock's packed input up front: the whole
            # input stream (2.6 MB) fits in SBUF and keeps the sync HWDGE
            # queue busy from the first microsecond.
            wts = []
            for s in range(NSUP):
                wt = win_pool.tile([128, WIN_BYTES], u8)
                nc.sync.dma_start(out=wt, in_=win_d[s])
                wts.append(wt)

            for s in range(NSUP):
                wt = wts[s]
                wh_v = wt[:, 0:1024].bitcast(f16)          # [128, 512]
                wl_v = wt[:, 1024:2048].bitcast(f16)       # [128, 512]
                sh_v = wt[:, 2048:2560].bitcast(f32)       # [128, 128]

                u_t = u_pool.tile([128, FB], f32, tag="uf")
                # y = (t_rel + (t0 - c_j))^2
                nc.scalar.activation(
                    u_t, tt_v, Act.Square, bias=negc_v[:, s : s + 1], scale=1.0
                )
                # y -= (t - c_nearest)^2  (per-frame max-energy shift)
                nc.vector.tensor_sub(u_t, u_t, sh_v)
                # u = exp(-DELTA * y)
                nc.scalar.activation(u_t, u_t, Act.Exp, scale=-DELTA)
                u16_t = u_pool.tile([128, FB], f16, tag="u16")
                nc.vector.tensor_copy(u16_t, u_t)

                ps = pss_pool.tile([128, GRP], f32)
                nc.tensor.matmul(ps, lhsT=u16_t, rhs=g_v, start=True, stop=True)
                r_t = rc_pool.tile([128, GRP], f32)
                nc.vector.reciprocal(r_t, ps)

                ob = out_pool.tile([128, GRP * ADIM], f32)
                for g in range(GRP):
                    blk = s * GRP + g
                    sl = slice(g * W, (g + 1) * W)
                    tp = (g * W, 0)
                    pm = psm_pool.tile([128, ADIM], f32)
                    nc.tensor.matmul(
                        pm, lhsT=u16_t[sl, :], rhs=wh_v[sl, :],
                        start=True, stop=False, tile_position=tp,
                    )
                    nc.tensor.matmul(
                        pm, lhsT=u16_t[sl, :], rhs=wl_v[sl, :],
                        start=False, stop=True, tile_position=tp,
                    )
                    dst = ob[:, g * ADIM : (g + 1) * ADIM]
                    if blk % 2 == 0:
                        nc.scalar.activation(
                            dst, pm, Act.Copy, scale=r_t[:, g : g + 1]
                        )
                    else:
                        nc.vector.tensor_scalar_mul(dst, pm, r_t[:, g : g + 1])
                # output DMA rides the gpsimd software DGE queue so it
                # overlaps the input stream on the sync HWDGE queue.
                nc.gpsimd.dma_start(out=out[s], in_=ob)

    nc.compile()
    return nc


def _host_prep(hs, ds):
    """Per-core input maps: packed gathered hs windows + bias tables."""
    hs = np.asarray(hs, dtype=np.float32)
    ds = np.asarray(ds)
    in_maps = []
    ttab = np.tile(np.arange(FB, dtype=np.float32), (128, 1))
    gm = np.zeros((128, GRP), dtype=np.float16)
    for g in range(GRP):
        gm[g * W : (g + 1) * W, g] = 1.0
    for b in range(B):
        ds_f = ds[b].astype(np.float32)
        c = np.cumsum(ds_f) - ds_f / 2.0  # token centers, fp32 as in reference
        # nearest center per output frame (for the max-energy shift)
        t_all = np.arange(T_FEATS, dtype=np.float32)
        ins_pt = np.searchsorted(c, t_all)
        cand_lo = np.clip(ins_pt - 1, 0, T_TEXT - 1)
        cand_hi = np.clip(ins_pt, 0, T_TEXT - 1)
        pick_hi = np.abs(c[cand_hi] - t_all) < np.abs(c[cand_lo] - t_all)
        near = np.where(pick_hi, cand_hi, cand_lo)
        d2 = (t_all - c[near]) ** 2  # fp32
        for q in range(Q_PER_B):
            hs_win = np.zeros((NSUP, 128, ADIM), dtype=np.float32)
            negc = np.zeros((128, NSUP), dtype=np.float32)
            shift = np.zeros((NSUP, 128, FB), dtype=np.float32)
            for s in range(NSUP):
                for g in range(GRP):
                    gi = q * NBLK + s * GRP + g  # global block in this batch
                    t0 = gi * FB
                    lo = int(np.searchsorted(c, t0 - REACH, side="left"))
                    hi = int(np.searchsorted(c, t0 + (FB - 1) + REACH, side="right"))
                    n_lo = int(near[t0 : t0 + FB].min())
                    n_hi = int(near[t0 : t0 + FB].max())
                    j0 = max(0, min(lo, n_lo, T_TEXT - W))
                    assert max(hi, n_hi + 1) - j0 <= W, (
                        f"token window {max(hi, n_hi + 1) - j0} exceeds {W}; "
                        "durations too small for this kernel's banding"
                    )
                    hs_win[s, g * W : (g + 1) * W, :] = hs[b, j0 : j0 + W, :]
                    negc[g * W : (g + 1) * W, s] = t0 - c[j0 : j0 + W]
                    shift[s, g * W : (g + 1) * W, :] = d2[t0 : t0 + FB]
            wh = hs_win.astype(np.float16)
            wl = (hs_win - wh.astype(np.float32)).astype(np.float16)
            win = np.empty((NSUP, 128, WIN_BYTES), dtype=np.uint8)
            win[:, :, 0:1024] = wh.view(np.uint8)
            win[:, :, 1024:2048] = wl.view(np.uint8)
            win[:, :, 2048:2560] = shift.view(np.uint8)
            con = np.empty((128, CON_BYTES), dtype=np.uint8)
            con[:, CON_TT : CON_TT + 512] = ttab.view(np.uint8)
            con[:, CON_NEGC : CON_NEGC + 4 * NSUP] = negc.view(np.uint8)
            con[:, CON_G : CON_G + 2 * GRP] = gm.view(np.uint8)
            in_maps.append({"win": win, "con": con})
    return in_maps


def kernel(hs, ds):
    global _LAST_EXEC_NS
    in_maps = _host_prep(hs, ds)
    nc = _build_program()

    kwargs = {}
    if os.environ.get("GU_TRACE") == "1":
        import concourse.bass_utils as bu

        bu.upload_artifacts = lambda tmpdir: "local://" + tmpdir
        kwargs = {"trace": True}
    res = run_bass_kernel_spmd(nc, in_maps, list(range(N_CORES)), **kwargs)
    _LAST_EXEC_NS = res.exec_time_ns

    full = np.empty((B, T_FEATS, ADIM), dtype=np.float32)
    for b in range(B):
        for q in range(Q_PER_B):
            core = b * Q_PER_B + q
            blocked = res.results[core]["out"]  # [NSUP, 128, GRP*ADIM]
            o = blocked.reshape(NSUP, 128, GRP, ADIM)
            o = o.transpose(0, 2, 1, 3).reshape(F_CORE, ADIM)
            full[b, q * F_CORE : (q + 1) * F_CORE, :] = o
    return full
